# revision 1
# baseline (speedup 1.0000x reference)
"""Trainium2 Bass kernel for nn_Critic (gnn_message_passing).

Strategy (8 NeuronCores, one SPMD NEFF):
  - Node-shard the per-node MLPs (8 nodes/core) so the big per-node weights
    (67MB total) are read once across the chip instead of replicated.
  - mm1 in [b, o] layout, LN stats via DVE bn_stats/bn_aggr, rsqrt as
    exp(-0.5*ln(var+eps)) (keeps every ACT function in one LUT table-set),
    PE transpose, fused scale/bias/relu on ACT, mm2 in [d, b] layout.
  - Q=A+V and V are packed [node, d, {q,v}, b] in bf16 and AllGathered in
    two halves so the first collective overlaps the second half of the MLP
    compute.
  - Choquet phase is set-sharded (8 sets/core; Q and V ride together):
    neighbor blocks fetched as contiguous 64KB register-offset HWDGE DMAs
    (offsets from an int32 input -> batched reg loads, alternating the
    sync/scalar DMA queues), pair terms via delta-batched DVE mins over
    two-set groups, and all d-reductions as PSUM-accumulated weighted
    matmuls on the Tensor engine (diagonal-block extraction).
All per-core-varying structure (gather offsets, Mobius coefficients) enters
as input data so the single SPMD program stays uniform across cores.
"""

import os

import numpy as np
import ml_dtypes

import concourse.bass as bass
import concourse.bacc as bacc
import concourse.mybir as mybir
from concourse import tile
from concourse.bass_utils import run_bass_kernel_spmd

DEBUG = bool(os.environ.get("KERNEL_DEBUG"))

B, N, H, D, K, HEADS = 128, 64, 256, 128, 8, 3
NCORE = 8
NLOC = N // NCORE      # nodes per core
SLOC = N // NCORE      # sets per core
NINST = 2 * SLOC       # choquet instances per core (Q sets then V sets)
NSLOT = K + 1          # center + 8 neighbors
NPAIR = (K * (K - 1)) // 2  # 28
NH = NLOC // 2         # nodes per collective half
F32 = mybir.dt.float32
BF16 = mybir.dt.bfloat16
I32 = mybir.dt.int32

# pairs in delta-major order over neighbor slots 1..8
PAIRS = [(a, a + d) for d in range(1, K) for a in range(1, K - d + 1)]

_compiled = None


def _build():
    nc = bacc.Bacc("TRN2", target_bir_lowering=False, debug=False,
                   num_devices=NCORE)

    # ---- per-core inputs ----
    obsT = nc.dram_tensor("obsT", [NLOC, H, B], F32, kind="ExternalInput")
    actT = nc.dram_tensor("actT", [NLOC, H, B], F32, kind="ExternalInput")
    # packed weights: wpV[i, p, c, :] = [W1V_chunk(256) | W2V_chunk(128) |
    # W2A_chunk(128)]; wpA[i, p, c, :] = W1A chunk (256)
    wpV = nc.dram_tensor("wpV", [NLOC, 128, 2, 512], F32,
                         kind="ExternalInput")
    wpA = nc.dram_tensor("wpA", [NLOC, 128, 4, 256], F32,
                         kind="ExternalInput")
    # packed biases: [b1V(256) | b2V(128) | b1A(256) | b2A(128)]
    bp = nc.dram_tensor("bp", [NLOC, 768], F32, kind="ExternalInput")
    lnV = nc.dram_tensor("lnV", [B, 4], F32, kind="ExternalInput")
    lnA = nc.dram_tensor("lnA", [B, 4], F32, kind="ExternalInput")
    # row-base offsets (into the half-split qvall) for each (set, slot)
    gbase = nc.dram_tensor("gbase", [1, SLOC * NSLOT], I32,
                           kind="ExternalInput")
    pw = nc.dram_tensor("pw", [128, SLOC, 7, 4], BF16, kind="ExternalInput")
    sw = nc.dram_tensor("sw", [128, SLOC, 3, 4], BF16, kind="ExternalInput")
    ident = nc.dram_tensor("ident", [128, 128], F32, kind="ExternalInput")

    chi = nc.dram_tensor("chi", [1, NINST * B], F32, kind="ExternalOutput")
    if DEBUG:
        dbg_x = nc.dram_tensor("dbg_x", [128, NSLOT, 2, B], BF16,
                               kind="ExternalOutput")
        dbg_pm = nc.dram_tensor("dbg_pm", [128, NPAIR, 2, B], BF16,
                                kind="ExternalOutput")

    with tile.TileContext(nc, num_cores=NCORE) as tc:
        with tc.tile_pool(name="const", bufs=1) as cpool, \
             tc.tile_pool(name="dram", bufs=1, space="DRAM") as dram:
            ident_s = cpool.tile([128, 128], F32)
            nc.sync.dma_start(out=ident_s[:], in_=ident[:])
            ones_row = cpool.tile([1, 128], F32)
            nc.vector.memset(ones_row[:], 1.0)
            eps_t = cpool.tile([B, 1], F32)
            nc.vector.memset(eps_t[:], 1e-5)
            lnV_s = cpool.tile([B, 4], F32)
            nc.sync.dma_start(out=lnV_s[:], in_=lnV[:])
            lnA_s = cpool.tile([B, 4], F32)
            nc.sync.dma_start(out=lnA_s[:], in_=lnA[:])
            pw_s = cpool.tile([128, SLOC, 7, 4], BF16)
            nc.sync.dma_start(out=pw_s[:], in_=pw[:])
            sw_s = cpool.tile([128, SLOC, 3, 4], BF16)
            nc.sync.dma_start(out=sw_s[:], in_=sw[:])
            gbase_s = cpool.tile([1, SLOC * NSLOT], I32)
            nc.sync.dma_start(out=gbase_s[:], in_=gbase[:])

            qvloc = dram.tile([NLOC, D, 2, B], BF16)
            # [core, node, d, {q,v}, b]
            qvall = dram.tile([NCORE, NLOC, D, 2, B], BF16,
                              addr_space="Shared")

            # HAM warm-up: a dense burst of junk matmuls at kernel start
            # (PE would otherwise idle while weights stream in) lifts the
            # PE clock gate to 8/8 before the real matmuls arrive.
            warm_rhs = cpool.tile([128, 512], F32)
            nc.vector.memset(warm_rhs[:], 0.0)
            with tc.tile_pool(name="ps_w", bufs=1, space="PSUM") as ps_w:
                wp = ps_w.tile([128, 512], F32)
                for k in range(10):
                    nc.tensor.matmul(wp[:], ident_s[:], warm_rhs[:],
                                     start=(k == 0), stop=(k == 9))

            # ================= Phase 1: per-node MLPs =================
            with tc.tile_pool(name="p1", bufs=3) as p1, \
                 tc.tile_pool(name="p1w", bufs=4) as p1w, \
                 tc.tile_pool(name="ps_h", bufs=3, space="PSUM") as ps_h, \
                 tc.tile_pool(name="ps_t", bufs=2, space="PSUM") as ps_t, \
                 tc.tile_pool(name="ps_o", bufs=2, space="PSUM") as ps_o:

                def mlp(xt_tiles, w1_aps, b1_ap, ln_s, w2_aps, b2_ap):
                    # mm1: psum_h[b, o]
                    h = ps_h.tile([B, H], F32, tag="h")
                    for c, (xt, w1c) in enumerate(zip(xt_tiles, w1_aps)):
                        nc.tensor.matmul(h[:], xt, w1c,
                                         start=(c == 0), stop=False)
                    nc.tensor.matmul(h[:], ones_row[:], b1_ap,
                                     start=False, stop=True)
                    # LN stats via bn_stats/bn_aggr -> [mu, var]
                    bn6 = p1.tile([B, 6], F32, tag="bn6")
                    nc.vector.bn_stats(bn6[:], h[:])
                    bn2 = p1.tile([B, 2], F32, tag="bn2")
                    nc.vector.bn_aggr(bn2[:], bn6[:])
                    # rs = 1/sqrt(var+eps) = exp(-0.5*ln(var+eps))
                    lv = p1.tile([B, 1], F32, tag="lv")
                    nc.scalar.activation(lv[:], bn2[:, 1:2],
                                         mybir.ActivationFunctionType.Ln,
                                         bias=eps_t[:])
                    rs = p1.tile([B, 1], F32, tag="rs")
                    nc.scalar.activation(rs[:], lv[:],
                                         mybir.ActivationFunctionType.Exp,
                                         scale=-0.5)
                    # apply (h-mu)*rs
                    u = p1.tile([B, H], F32, tag="u")
                    nc.vector.tensor_scalar(u[:], h[:], bn2[:, 0:1], rs[:],
                                            mybir.AluOpType.subtract,
                                            mybir.AluOpType.mult)
                    # transpose u -> uT; hT = relu(g*uT + be) on DVE
                    ut = ps_t.tile([128, 2, 128], F32, tag="ut")
                    for c in range(2):
                        nc.tensor.transpose(ut[:, c, :],
                                            u[:, c * 128:(c + 1) * 128],
                                            ident_s[:])
                    hT = p1.tile([128, 2, 128], F32, tag="hT")
                    for c in range(2):
                        nc.vector.tensor_scalar(
                            hT[:, c, :], ut[:, c, :],
                            ln_s[:, c:c + 1], ln_s[:, 2 + c:3 + c],
                            mybir.AluOpType.mult, mybir.AluOpType.add)
                        nc.vector.tensor_scalar(
                            hT[:, c, :], hT[:, c, :], 0.0, None,
                            mybir.AluOpType.max)
                    # mm2: out[d, b]
                    o = ps_o.tile([D, B], F32, tag="o")
                    for c in range(2):
                        nc.tensor.matmul(o[:], w2_aps[c], hT[:, c, :],
                                         start=(c == 0), stop=False)
                    nc.tensor.matmul(o[:], b2_ap, ones_row[:],
                                     start=False, stop=True)
                    return o

                for i in range(NLOC):
                    xv = p1.tile([128, 2, B], F32, tag="xv")
                    nc.gpsimd.dma_start(
                        out=xv[:],
                        in_=obsT[i].rearrange("(c p) b -> p c b", p=128))
                    xa = p1.tile([128, 2, B], F32, tag="xa")
                    nc.gpsimd.dma_start(
                        out=xa[:],
                        in_=actT[i].rearrange("(c p) b -> p c b", p=128))
                    wv = p1w.tile([128, 2, 512], F32, tag="wv")
                    nc.sync.dma_start(out=wv[:], in_=wpV[i])
                    wa = p1w.tile([128, 4, 256], F32, tag="wa")
                    nc.scalar.dma_start(out=wa[:], in_=wpA[i])
                    bt = p1w.tile([1, 768], F32, tag="bt")
                    nc.gpsimd.dma_start(out=bt[:], in_=bp[i][None, :])
                    ov = mlp([xv[:, 0, :], xv[:, 1, :]],
                             [wv[:, 0, 0:256], wv[:, 1, 0:256]],
                             bt[:, 0:256], lnV_s,
                             [wv[:, 0, 256:384], wv[:, 1, 256:384]],
                             bt[:, 256:384])
                    oa = mlp([xv[:, 0, :], xv[:, 1, :],
                              xa[:, 0, :], xa[:, 1, :]],
                             [wa[:, 0, :], wa[:, 1, :],
                              wa[:, 2, :], wa[:, 3, :]],
                             bt[:, 384:640], lnA_s,
                             [wv[:, 0, 384:512], wv[:, 1, 384:512]],
                             bt[:, 640:768])
                    vf = p1.tile([D, B], F32, tag="vf")
                    nc.scalar.copy(vf[:], ov[:])
                    qb = p1.tile([D, B], BF16, tag="qb")
                    nc.vector.tensor_tensor(qb[:], oa[:], vf[:],
                                            mybir.AluOpType.add)
                    vb = p1.tile([D, B], BF16, tag="vb")
                    nc.vector.tensor_scalar(vb[:], vf[:], 1.0, None,
                                            mybir.AluOpType.mult)
                    nc.sync.dma_start(out=qvloc[i, :, 0, :], in_=qb[:])
                    nc.scalar.dma_start(out=qvloc[i, :, 1, :], in_=vb[:])

            # ================= Phase 2: AllGather =================
            nc.gpsimd.collective_compute(
                "AllGather", mybir.AluOpType.bypass,
                replica_groups=[list(range(NCORE))],
                ins=[qvloc.opt()], outs=[qvall.opt()],
            )
            # flat rows: row = node*128 + d, each [2*B]
            qv_flat = qvall.rearrange("c n p t b -> (c n p) (t b)")

            # ================= Phase 3: Choquet =================
            SG = 2  # sets per group (shared X/PM tiles)
            with tc.tile_pool(name="p3", bufs=3) as p3, \
                 tc.tile_pool(name="p3pm", bufs=2) as p3pm, \
                 tc.tile_pool(name="ps_p", bufs=4, space="PSUM") as ps_p, \
                 tc.tile_pool(name="ps_r", bufs=1, space="PSUM") as ps_r, \
                 tc.tile_pool(name="p3c", bufs=1) as p3c:
                chirow = ps_r.tile([1, NINST * B], F32)
                chi4 = [p3c.tile([4, SLOC, 4, B], F32,
                                 name="chi4q", tag="chi4q"),
                        p3c.tile([4, SLOC, 4, B], F32,
                                 name="chi4v", tag="chi4v")]
                for g in range(SLOC // SG):
                    X = p3.tile([128, SG, NSLOT, 2, B], BF16, tag="X")
                    # contiguous 64KB block DMAs with register offsets,
                    # alternating the two HWDGE queues (sync / scalar)
                    eng_t, eng = ((mybir.EngineType.SP, nc.sync)
                                  if g % 2 == 0 else
                                  (mybir.EngineType.Activation, nc.scalar))
                    j0 = g * SG * NSLOT
                    _, vals = nc.values_load_multi_w_load_instructions(
                        gbase_s[0:1, j0:j0 + SG * NSLOT],
                        engines=[eng_t],
                        skip_runtime_bounds_check=True)
                    for sl in range(SG):
                        for k in range(NSLOT):
                            eng.dma_start(
                                out=X[:, sl, k, :, :],
                                in_=qv_flat[
                                    bass.ds(vals[sl * NSLOT + k], 128), :])
                    if g == 0:
                        # re-warm the PE clock gate while the first gathers
                        # and mins are still in flight (junk matmuls on X)
                        wp3 = ps_p.tile([4, 4 * B], F32, tag="P")
                        for k in range(10):
                            nc.tensor.matmul(wp3[:], pw_s[:, 0, 0, :],
                                             X[:, 0, 0:4, 0, :],
                                             start=(k == 0), stop=(k == 9))
                    PM = p3pm.tile([128, SG, NPAIR, 2, B], BF16, tag="PM")
                    off = 0
                    for dd in range(1, K):
                        n = K - dd
                        nc.vector.tensor_tensor(
                            PM[:, :, off:off + n, :, :],
                            X[:, :, 1:1 + n, :, :],
                            X[:, :, 1 + dd:1 + dd + n, :, :],
                            mybir.AluOpType.min)
                        off += n
                    if DEBUG and g == 0:
                        nc.sync.dma_start(out=dbg_x[:], in_=X[:, 0])
                        nc.sync.dma_start(out=dbg_pm[:], in_=PM[:, 0])
                    for sl in range(SG):
                        s = g * SG + sl
                        for t in range(2):
                            P = ps_p.tile([4, 4 * B], F32, tag="P")
                            for j in range(7):
                                nc.tensor.matmul(
                                    P[:], pw_s[:, s, j, :],
                                    PM[:, sl, 4 * j:4 * j + 4, t, :],
                                    start=(j == 0), stop=False)
                            for j, s0 in enumerate((0, 4, 5)):
                                nc.tensor.matmul(
                                    P[:], sw_s[:, s, j, :],
                                    X[:, sl, s0:s0 + 4, t, :],
                                    start=False, stop=(j == 2))
                            dst = chi4[t][:, s, :, :].rearrange(
                                "p a b -> p (a b)")
                            nc.scalar.copy(dst, P[:])
                # fold: extract diagonal blocks P[c, c, b]; batch 4 sets/MM
                for t in range(2):
                    for g4 in range(SLOC // 4):
                        for c in range(4):
                            nc.tensor.matmul(
                                chirow[:, (t * SLOC + g4 * 4) * B:
                                       (t * SLOC + g4 * 4 + 4) * B],
                                ident_s[:4, c:c + 1],
                                chi4[t][:, 4 * g4:4 * g4 + 4, c, :],
                                start=(c == 0), stop=(c == 3))
                chirow_s = p3c.tile([1, NINST * B], F32)
                nc.scalar.copy(chirow_s[:], chirow[:])
                nc.sync.dma_start(out=chi[:], in_=chirow_s[:])

    nc.compile()
    return nc


def _prepare_inputs(observation, action, local_edges, V_W1, V_b1, V_g1,
                    V_beta1, V_W2, V_b2, A_W1, A_b1, A_g1, A_beta1, A_W2,
                    A_b2, chi_m1, chi_m2):
    centers = np.asarray(local_edges[:, 0, 0]).astype(np.int64)
    neigh = np.asarray(local_edges[:, 0, 1:]).astype(np.int64)
    m1s = chi_m1.sum(1) / (HEADS * D)              # [S, K]
    tri = np.triu(np.ones((K, K), np.float32), k=1)
    m2s = (chi_m2.sum(1) * tri) / (HEADS * D)      # [S, K, K]

    in_maps = []
    for c in range(NCORE):
        nodes = slice(c * NLOC, (c + 1) * NLOC)
        obsn = observation[:, nodes, :]            # [B, 8, H]
        actn = action[:, nodes, :]
        m = {}
        m["obsT"] = np.ascontiguousarray(obsn.transpose(1, 2, 0))
        m["actT"] = np.ascontiguousarray(actn.transpose(1, 2, 0))
        # packed weights: wpV[i, p, c, :] = [W1V[c*128+p, :256] |
        #                                    W2V[c*128+p, :] | W2A[c*128+p, :]]
        w1v = V_W1[nodes].reshape(NLOC, 2, 128, H).transpose(0, 2, 1, 3)
        w2v = V_W2[nodes].reshape(NLOC, 2, 128, D).transpose(0, 2, 1, 3)
        w2a = A_W2[nodes].reshape(NLOC, 2, 128, D).transpose(0, 2, 1, 3)
        m["wpV"] = np.ascontiguousarray(
            np.concatenate([w1v, w2v, w2a], axis=3))
        m["wpA"] = np.ascontiguousarray(
            A_W1[nodes].reshape(NLOC, 4, 128, H).transpose(0, 2, 1, 3))
        m["bp"] = np.ascontiguousarray(np.concatenate(
            [V_b1[nodes], V_b2[nodes], A_b1[nodes], A_b2[nodes]], axis=1))
        lnv = np.zeros((B, 4), np.float32)
        lnv[:, 0] = V_g1[:128]; lnv[:, 1] = V_g1[128:]
        lnv[:, 2] = V_beta1[:128]; lnv[:, 3] = V_beta1[128:]
        m["lnV"] = lnv
        lna = np.zeros((B, 4), np.float32)
        lna[:, 0] = A_g1[:128]; lna[:, 1] = A_g1[128:]
        lna[:, 2] = A_beta1[:128]; lna[:, 3] = A_beta1[128:]
        m["lnA"] = lna

        gb = np.zeros((1, SLOC * NSLOT), np.int32)
        pwn = np.zeros((SLOC, 7, 4), np.float32)
        swn = np.zeros((SLOC, 3, 4), np.float32)
        for sl in range(SLOC):
            s = c * SLOC + sl
            slots = [int(centers[s])] + [int(x) for x in neigh[s]]
            for k in range(NSLOT):
                gb[0, sl * NSLOT + k] = slots[k] * D
            for p, (a, b_) in enumerate(PAIRS):
                pwn[sl, p // 4, p % 4] = m2s[s, a - 1, b_ - 1]
            # device slot groups: j=0 slots 0-3, j=1 slots 4-7, j=2 slots 5-8
            swn[sl, 0, 0] = 1.0 / D                # center
            for k in range(1, 8):
                swn[sl, k // 4, k % 4] = m1s[s, k - 1]
            swn[sl, 2, 3] = m1s[s, 7]              # slot 8
        m["gbase"] = gb
        m["pw"] = np.broadcast_to(
            pwn.astype(ml_dtypes.bfloat16)[None], (128, SLOC, 7, 4)).copy()
        m["sw"] = np.broadcast_to(
            swn.astype(ml_dtypes.bfloat16)[None], (128, SLOC, 3, 4)).copy()
        m["ident"] = np.eye(128, dtype=np.float32)
        in_maps.append(m)
    return in_maps


def kernel(**inputs):
    global _compiled
    if _compiled is None:
        _compiled = _build()
    nc = _compiled
    inputs = {k: np.asarray(v) for k, v in inputs.items()}
    in_maps = _prepare_inputs(**inputs)
    res = run_bass_kernel_spmd(nc, in_maps, list(range(NCORE)))
    global _last_results
    _last_results = res
    chi_q = np.zeros((B, N), np.float32)
    chi_v = np.zeros((B, N), np.float32)
    for c in range(NCORE):
        out = res.results[c]["chi"].reshape(NINST, B)
        for sl in range(SLOC):
            chi_q[:, c * SLOC + sl] = out[sl]
            chi_v[:, c * SLOC + sl] = out[SLOC + sl]
    return chi_q, chi_v



# revision 21
# speedup vs baseline: 1.4867x; 1.4867x over previous
"""Trainium2 Bass kernel for nn_Critic (gnn_message_passing) — v2.

Strategy (8 NeuronCores, one SPMD NEFF):
  Phase 1 (node-sharded, 8 nodes/core): per-node MLPs fully in bf16 on PE
    (1 cyc/col vs 4 for fp32). LN stats via DVE bn_stats/bn_aggr; rsqrt =
    ACT Sqrt(var+eps) + DVE reciprocal; the per-chunk scale/bias/relu is a
    single ACT Relu(g*x + be) with per-partition scale/bias. Every LUT
    function used (Sqrt/Relu/Copy) lives in the one 'sqrt_and_others'
    table -> a single ACT_TABLE_LOAD for the whole kernel.
    Per node we also emit d-sums of Q and V (ones-matmul) so the Choquet
    singles+center terms can be formed from a tiny [64, b] tensor later.
  Phase 2: AllGather in TWO chunks (nodes 0-3 | nodes 4-7 + d-sums) with
    chunk-major DRAM layout so both collectives use contiguous APs; chunk 0
    overlaps the second half of phase-1 compute.
  Phase 3 (set-sharded, 8 sets/core): X gathered with contiguous 64KB
    register-offset HWDGE DMAs alternating the sync/scalar queues; pairmin
    via delta-batched DVE mins (large deltas on the Pool engine); the
    weighted d+pair reduction on PE via diagonal-block matmuls [4, 4B];
    the singles+center ride as one extra accumulated matmul per (set,t)
    whose off-diagonal garbage lands in fold-discarded entries; fold =
    512-col matmuls per (t, half) that pipeline with the last groups.
All per-core-varying structure (gather offsets, Mobius coefficients)
enters as input data so the single SPMD program stays uniform.
"""

import numpy as np
import ml_dtypes

import concourse.bass as bass
import concourse.bacc as bacc
import concourse.mybir as mybir
from concourse import tile
from concourse.bass_utils import run_bass_kernel_spmd

B, N, H, D, K, HEADS = 128, 64, 256, 128, 8, 3
NCORE = 8
NLOC = N // NCORE      # nodes per core
SLOC = N // NCORE      # sets per core
NINST = 2 * SLOC       # (set, t) instances per core
NSLOT = K + 1          # center + 8 neighbors
NPAIR = (K * (K - 1)) // 2  # 28
CH_ROWS = 4 * D + NLOC  # rows per AllGather chunk: 4 nodes + 8 sum rows
F32 = mybir.dt.float32
BF16 = mybir.dt.bfloat16
I32 = mybir.dt.int32

# pairs in delta-major order over neighbor slots 1..8
PAIRS = [(a, a + d) for d in range(1, K) for a in range(1, K - d + 1)]

_compiled = None


def _build():
    nc = bacc.Bacc("TRN2", target_bir_lowering=False, debug=False,
                   num_devices=NCORE)

    # ---- per-core inputs (host-packed) ----
    xin = nc.dram_tensor("xin", [NLOC, 128, 4, B], BF16, kind="ExternalInput")
    wp = nc.dram_tensor("wp", [NLOC, 128, 2048], BF16, kind="ExternalInput")
    bia = nc.dram_tensor("bia", [1, NLOC * 768], BF16, kind="ExternalInput")
    lnw = nc.dram_tensor("lnw", [128, 8], F32, kind="ExternalInput")
    pw = nc.dram_tensor("pw", [128, SLOC, 7, 4], BF16, kind="ExternalInput")
    ws = nc.dram_tensor("ws", [64, SLOC, 4], BF16, kind="ExternalInput")
    gb = nc.dram_tensor("gb", [1, SLOC * NSLOT], I32, kind="ExternalInput")
    ident = nc.dram_tensor("ident", [128, 128], BF16, kind="ExternalInput")

    chi = nc.dram_tensor("chi", [1, NINST * B], F32, kind="ExternalOutput")

    with tile.TileContext(nc, num_cores=NCORE) as tc:
        with tc.tile_pool(name="const", bufs=1) as cpool, \
             tc.tile_pool(name="dram", bufs=1, space="DRAM") as dram:
            ident_s = cpool.tile([128, 128], BF16)
            nc.sync.dma_start(out=ident_s[:], in_=ident[:])
            ones_row = cpool.tile([1, 128], BF16)
            nc.vector.memset(ones_row[:], 1.0)
            ones_col = cpool.tile([128, 1], BF16)
            nc.vector.memset(ones_col[:], 1.0)
            eps_t = cpool.tile([B, 1], F32)
            nc.vector.memset(eps_t[:], 1e-5)
            lnw_s = cpool.tile([128, 8], F32)
            nc.sync.dma_start(out=lnw_s[:], in_=lnw[:])
            pw_s = cpool.tile([128, SLOC, 7, 4], BF16)
            nc.scalar.dma_start(out=pw_s[:], in_=pw[:])
            ws_s = cpool.tile([64, SLOC, 4], BF16)
            nc.scalar.dma_start(out=ws_s[:], in_=ws[:])
            bia_s = cpool.tile([1, NLOC * 768], BF16)
            nc.gpsimd.dma_start(out=bia_s[:], in_=bia[:])
            gb_s = cpool.tile([1, SLOC * NSLOT], I32)
            nc.sync.dma_start(out=gb_s[:], in_=gb[:])
            warm_rhs = cpool.tile([128, 512], BF16)
            nc.vector.memset(warm_rhs[:], 0.0)
            # node 7's q/v lives in a persistent tile so the phase-3
            # PE keep-warm can read it after the phase-1 pools close
            qvb_last = cpool.tile([D, 2, B], BF16)

            # chunk-major gathered buffer:
            #   [chunk h, core, row, 2B] ; rows within a chunk:
            #   h=0: nodes 0-3 ([node, d] -> i*128+d), rows 512..520 pad
            #   h=1: nodes 4-7, rows 512..520 = per-node [sumQ | sumV]
            qvloc = dram.tile([2, CH_ROWS, 2 * B], BF16)
            qvall = dram.tile([NCORE, 2, CH_ROWS, 2 * B], BF16,
                              addr_space="Shared")
            qv_flat = qvall.rearrange("c h r w -> (c h r) w")

            # HAM warm-up: lift the PE clock gate before real matmuls.
            with tc.tile_pool(name="ps_w", bufs=1, space="PSUM") as ps_w:
                wpp = ps_w.tile([128, 512], F32)
                for k in range(10):
                    nc.tensor.matmul(wpp[:], ident_s[:], warm_rhs[:],
                                     start=(k == 0), stop=(k == 9))

            # ================= Phase 1: per-node MLPs =================
            with tc.tile_pool(name="p1", bufs=3) as p1, \
                 tc.tile_pool(name="p1w", bufs=2) as p1w, \
                 tc.tile_pool(name="p1s", bufs=1) as p1s, \
                 tc.tile_pool(name="ps_h", bufs=2, space="PSUM") as ps_h, \
                 tc.tile_pool(name="ps_t", bufs=2, space="PSUM") as ps_t, \
                 tc.tile_pool(name="ps_o", bufs=2, space="PSUM") as ps_o, \
                 tc.tile_pool(name="ps_s", bufs=2, space="PSUM") as ps_s:

                sums8 = p1s.tile([1, NLOC * 2 * B], BF16)

                for i in range(NLOC):
                    xt = p1.tile([128, 4, B], BF16, tag="xt")
                    nc.sync.dma_start(out=xt[:], in_=xin[i])
                    wt = p1w.tile([128, 2048], BF16, tag="wt")
                    nc.scalar.dma_start(out=wt[:], in_=wp[i])
                    bo = i * 768

                    # mm1 for both branches: h[b, o] in PSUM f32
                    h = ps_h.tile([B, 2, H], F32, tag="h")
                    for c in range(2):
                        nc.tensor.matmul(h[:, 0, :], xt[:, c, :],
                                         wt[:, c * 256:(c + 1) * 256],
                                         start=(c == 0), stop=False)
                    nc.tensor.matmul(h[:, 0, :], ones_row[:],
                                     bia_s[:, bo:bo + 256],
                                     start=False, stop=True)
                    for c in range(4):
                        nc.tensor.matmul(h[:, 1, :], xt[:, c, :],
                                         wt[:, 512 + c * 256:768 + c * 256],
                                         start=(c == 0), stop=False)
                    nc.tensor.matmul(h[:, 1, :], ones_row[:],
                                     bia_s[:, bo + 256:bo + 512],
                                     start=False, stop=True)

                    # LN + relu per branch, then mm2
                    ut = ps_t.tile([128, 2, 2, 128], BF16, tag="ut")
                    hT = p1.tile([128, 2, 2, 128], BF16, tag="hT")
                    o = ps_o.tile([D, 2, B], F32, tag="o")
                    for br in range(2):
                        bn6 = p1.tile([B, 6], F32, tag="bn6")
                        nc.vector.bn_stats(bn6[:], h[:, br, :])
                        bn2 = p1.tile([B, 2], F32, tag="bn2")
                        nc.vector.bn_aggr(bn2[:], bn6[:])
                        sd = p1.tile([B, 1], F32, tag="sd")
                        nc.scalar.activation(
                            sd[:], bn2[:, 1:2],
                            mybir.ActivationFunctionType.Sqrt,
                            bias=eps_t[:])
                        rs = p1.tile([B, 1], F32, tag="rs")
                        nc.vector.reciprocal(rs[:], sd[:])
                        # u = (h - mu) * rs
                        u = p1.tile([B, H], BF16, tag="u")
                        nc.vector.tensor_scalar(u[:], h[:, br, :],
                                                bn2[:, 0:1], rs[:],
                                                mybir.AluOpType.subtract,
                                                mybir.AluOpType.mult)
                        for c in range(2):
                            nc.tensor.transpose(ut[:, br, c, :],
                                                u[:, c * 128:(c + 1) * 128],
                                                ident_s[:])
                            # hT = relu(g * ut + be), one ACT op
                            nc.scalar.activation(
                                hT[:, br, c, :], ut[:, br, c, :],
                                mybir.ActivationFunctionType.Relu,
                                scale=lnw_s[:, 4 * br + c:4 * br + c + 1],
                                bias=lnw_s[:, 4 * br + 2 + c:4 * br + 3 + c])
                        w2o = 1536 + br * 256
                        for c in range(2):
                            nc.tensor.matmul(
                                o[:, br, :],
                                wt[:, w2o + c * 128:w2o + (c + 1) * 128],
                                hT[:, br, c, :],
                                start=(c == 0), stop=False)
                        nc.tensor.matmul(
                            o[:, br, :],
                            bia_s[:, bo + 512 + br * 128:bo + 640 + br * 128],
                            ones_row[:],
                            start=False, stop=True)

                    if i == NLOC - 1:
                        qvb = qvb_last
                    else:
                        qvb = p1.tile([D, 2, B], BF16, tag="qvb")
                    # v = copy oV (ACT), then q = oA + v (PSUM + SBUF)
                    nc.scalar.copy(qvb[:, 1, :], o[:, 0, :])
                    nc.vector.tensor_tensor(qvb[:, 0, :], o[:, 1, :],
                                            qvb[:, 1, :],
                                            mybir.AluOpType.add)
                    # d-sums of q and v -> sums8 row i
                    sq = ps_s.tile([1, 2 * B], F32, tag="sq")
                    nc.tensor.matmul(sq[:, 0:B], ones_col[:], qvb[:, 0, :],
                                     start=True, stop=True)
                    nc.tensor.matmul(sq[:, B:2 * B], ones_col[:],
                                     qvb[:, 1, :], start=True, stop=True)
                    nc.scalar.copy(
                        sums8[:, i * 2 * B:(i + 1) * 2 * B], sq[:])
                    # store node rows into its chunk
                    hh, i4 = (0, i) if i < 4 else (1, i - 4)
                    nc.sync.dma_start(
                        out=qvloc[hh, i4 * 128:(i4 + 1) * 128, :],
                        in_=qvb[:].rearrange("p t b -> p (t b)"))

                nc.gpsimd.dma_start(
                    out=qvloc[1, 512:512 + NLOC, :],
                    in_=sums8[:].rearrange("o (i w) -> (o i) w", w=2 * B))

            # ================= Phase 2: AllGather =====================
            nc.gpsimd.collective_compute(
                "AllGather", mybir.AluOpType.bypass,
                replica_groups=[list(range(NCORE))],
                ins=[qvloc.opt()], outs=[qvall.opt()],
            )

            # ================= Phase 3: Choquet =======================
            SG = 2  # sets per group
            NG = SLOC // SG
            with tc.tile_pool(name="p3", bufs=3) as p3, \
                 tc.tile_pool(name="p3pm", bufs=2) as p3pm, \
                 tc.tile_pool(name="p3c", bufs=1) as p3c, \
                 tc.tile_pool(name="ps_p", bufs=3, space="PSUM") as ps_p, \
                 tc.tile_pool(name="ps_r", bufs=1, space="PSUM") as ps_r:

                # d-sums for all 64 nodes -> [64, 2, B], replicated x4
                sq_all = p3c.tile([64, 2, B], BF16)
                nc.sync.dma_start(
                    out=sq_all[:],
                    in_=qvall[:, 1, 512:512 + NLOC, :])
                sq_rep = p3c.tile([64, 2, 4, B], BF16)
                for r in range(4):
                    nc.vector.tensor_copy(sq_rep[:, :, r, :], sq_all[:])

                # PE keep-warm during the AllGather gap (reads node-7 qvb
                # so the scheduler places it after phase-1 compute).
                with tc.tile_pool(name="ps_w2", bufs=1, space="PSUM") as psw:
                    wp2 = psw.tile([128, 256], F32)
                    for k in range(24):
                        nc.tensor.matmul(
                            wp2[:], ident_s[:],
                            qvb_last[:].rearrange("p t b -> p (t b)"),
                            start=(k == 0), stop=(k == 23))

                chi4 = p3c.tile([4, 2, SLOC, 4, B], BF16)
                chirow = ps_r.tile([1, NINST * B], F32)

                for g in range(NG):
                    X = p3.tile([128, SG, NSLOT, 2, B], BF16, tag="X")
                    eng_t, eng = ((mybir.EngineType.SP, nc.sync)
                                  if g % 2 == 0 else
                                  (mybir.EngineType.Activation, nc.scalar))
                    j0 = g * SG * NSLOT
                    _, vals = nc.values_load_multi_w_load_instructions(
                        gb_s[0:1, j0:j0 + SG * NSLOT],
                        engines=[eng_t],
                        skip_runtime_bounds_check=True)
                    for sl in range(SG):
                        for k in range(NSLOT):
                            eng.dma_start(
                                out=X[:, sl, k, :, :],
                                in_=qv_flat[
                                    bass.ds(vals[sl * NSLOT + k], 128), :])

                    PM = p3pm.tile([128, SG, NPAIR, 2, B], BF16, tag="PM")
                    off = 0
                    for dd in range(1, K):
                        n = K - dd
                        nc.vector.tensor_tensor(
                            PM[:, :, off:off + n, :, :],
                            X[:, :, 1:1 + n, :, :],
                            X[:, :, 1 + dd:1 + dd + n, :, :],
                            mybir.AluOpType.min)
                        off += n

                    for sl in range(SG):
                        s = g * SG + sl
                        for t in range(2):
                            P = ps_p.tile([4, 4 * B], F32, tag="P")
                            for j in range(7):
                                nc.tensor.matmul(
                                    P[:], pw_s[:, s, j, :],
                                    PM[:, sl, 4 * j:4 * j + 4, t, :],
                                    start=(j == 0), stop=False)
                            # singles + center: off-diag pollution is
                            # discarded by the fold
                            nc.tensor.matmul(
                                P[:], ws_s[:, s, :], sq_rep[:, t, :, :],
                                start=False, stop=True)
                            nc.scalar.copy(
                                chi4[:, t, s, :, :].rearrange(
                                    "p a b -> p (a b)"), P[:])

                    # fold half as soon as its 4 sets are done
                    if g % 2 == 1:
                        hh = g // 2
                        for t in range(2):
                            dst = chirow[:, t * SLOC * B + hh * 4 * B:
                                         t * SLOC * B + (hh + 1) * 4 * B]
                            for c in range(4):
                                nc.tensor.matmul(
                                    dst, ident_s[0:4, c:c + 1],
                                    chi4[:, t, 4 * hh:4 * hh + 4, c, :],
                                    start=(c == 0), stop=(c == 3))

                chirow_s = p3c.tile([1, NINST * B], F32)
                nc.scalar.copy(chirow_s[:], chirow[:])
                nc.sync.dma_start(out=chi[:], in_=chirow_s[:])

    nc.compile()
    return nc


def _prepare_inputs(observation, action, local_edges, V_W1, V_b1, V_g1,
                    V_beta1, V_W2, V_b2, A_W1, A_b1, A_g1, A_beta1, A_W2,
                    A_b2, chi_m1, chi_m2):
    bf16 = ml_dtypes.bfloat16
    centers = np.asarray(local_edges[:, 0, 0]).astype(np.int64)
    neigh = np.asarray(local_edges[:, 0, 1:]).astype(np.int64)
    m1s = chi_m1.sum(1) / (HEADS * D)              # [S, K]
    tri = np.triu(np.ones((K, K), np.float32), k=1)
    m2s = (chi_m2.sum(1) * tri) / (HEADS * D)      # [S, K, K]

    lnw = np.zeros((128, 8), np.float32)
    lnw[:, 0] = V_g1[:128];    lnw[:, 1] = V_g1[128:]
    lnw[:, 2] = V_beta1[:128]; lnw[:, 3] = V_beta1[128:]
    lnw[:, 4] = A_g1[:128];    lnw[:, 5] = A_g1[128:]
    lnw[:, 6] = A_beta1[:128]; lnw[:, 7] = A_beta1[128:]

    in_maps = []
    for c in range(NCORE):
        nodes = slice(c * NLOC, (c + 1) * NLOC)
        m = {}
        obs_n = observation[:, nodes, :].transpose(1, 2, 0)  # [8, H, B]
        act_n = action[:, nodes, :].transpose(1, 2, 0)
        xin = np.concatenate(
            [obs_n.reshape(NLOC, 2, 128, B).transpose(0, 2, 1, 3),
             act_n.reshape(NLOC, 2, 128, B).transpose(0, 2, 1, 3)],
            axis=2)                                           # [8,128,4,B]
        m["xin"] = np.ascontiguousarray(xin).astype(bf16)

        w1v = V_W1[nodes].reshape(NLOC, 2, 128, H).transpose(0, 2, 1, 3)
        w1a = A_W1[nodes].reshape(NLOC, 4, 128, H).transpose(0, 2, 1, 3)
        w2v = V_W2[nodes].reshape(NLOC, 2, 128, D).transpose(0, 2, 1, 3)
        w2a = A_W2[nodes].reshape(NLOC, 2, 128, D).transpose(0, 2, 1, 3)
        m["wp"] = np.ascontiguousarray(np.concatenate(
            [w1v.reshape(NLOC, 128, 512),
             w1a.reshape(NLOC, 128, 1024),
             w2v.reshape(NLOC, 128, 256),
             w2a.reshape(NLOC, 128, 256)], axis=2)).astype(bf16)
        m["bia"] = np.ascontiguousarray(np.concatenate(
            [V_b1[nodes], A_b1[nodes], V_b2[nodes], A_b2[nodes]],
            axis=1).reshape(1, NLOC * 768)).astype(bf16)
        m["lnw"] = lnw

        pwn = np.zeros((SLOC, 7, 4), np.float32)
        wsn = np.zeros((64, SLOC, 4), np.float32)
        gbn = np.zeros((1, SLOC * NSLOT), np.int32)
        for sl in range(SLOC):
            s = c * SLOC + sl
            for p, (a, b_) in enumerate(PAIRS):
                pwn[sl, p // 4, p % 4] = m2s[s, a - 1, b_ - 1]
            w = np.zeros(64, np.float32)
            for k in range(K):
                w[neigh[s, k]] += m1s[s, k]
            w[centers[s]] += 1.0 / D
            # each of the 4 fold-diagonal rows carries 1/4 of the singles
            wsn[:, sl, :] = w[:, None] / 4.0
            slots = [int(centers[s])] + [int(x) for x in neigh[s]]
            for k in range(NSLOT):
                g = slots[k]
                hh, cc, i4 = (g % NLOC) // 4, g // NLOC, (g % NLOC) % 4
                gbn[0, sl * NSLOT + k] = ((cc * 2 + hh) * CH_ROWS
                                          + i4 * 128)
        m["pw"] = np.broadcast_to(
            pwn.astype(bf16)[None], (128, SLOC, 7, 4)).copy()
        m["ws"] = wsn.astype(bf16)
        m["gb"] = gbn
        m["ident"] = np.eye(128, dtype=np.float32).astype(bf16)
        in_maps.append(m)
    return in_maps


def kernel(**inputs):
    global _compiled
    if _compiled is None:
        _compiled = _build()
    nc = _compiled
    inputs = {k: np.asarray(v) for k, v in inputs.items()}
    in_maps = _prepare_inputs(**inputs)
    res = run_bass_kernel_spmd(nc, in_maps, list(range(NCORE)))
    global _last_results
    _last_results = res
    chi_q = np.zeros((B, N), np.float32)
    chi_v = np.zeros((B, N), np.float32)
    for c in range(NCORE):
        out = res.results[c]["chi"].reshape(2, SLOC, B)
        for sl in range(SLOC):
            chi_q[:, c * SLOC + sl] = out[0, sl]
            chi_v[:, c * SLOC + sl] = out[1, sl]
    return chi_q, chi_v
